# revision 36
# baseline (speedup 1.0000x reference)
"""Multi-head attention (B=4, Q=K=2048, D=512, H=8) on 8 TRN2 NeuronCores.

Sharding: every core touches EVERY batch.  Core c owns head-pair hp = c%4
and query-half qh = c//4; its program runs 4 "groups", one per batch, and
group g's key extent is that batch's own ceil(valid_len/128)*128.  Per-core
work is therefore sum_b KT_b instead of 4*max_b KT_b -- perfectly balanced
across cores for any valid_lens.  Each group emits a partial output
projection (row-sharded W_o over its head pair); the host sums the 4
head-pair partials per (batch, query-half).

Device-side layout:
  * Activations live transposed ([feature, seq]); scores are computed
    transposed (S_T[k, q] = K_h @ Q_h^T) so the key-padding mask is a
    per-partition bias on the ACT exp and softmax needs no max pass.
  * The two heads of a pair sit at partitions 0-63 / 64-127 of shared
    q_t/k_t tiles, so their C=64 score matmuls are issued back-to-back and
    run CONCURRENTLY in disjoint PE row-groups (tile_position (0,0)/(64,0)
    auto-derived from base partitions).  Per chunk the scores land in two
    [128, 1024] PSUM tiles laid out [headA | headB] so each gets a single
    wide exp; the two tiles double-buffer in 4 PSUM banks (the other 4
    hold the attnV accumulators).
  * A ones-column interleaved into V makes the attnV matmul emit the
    softmax denominator for free (row 64 of each head's [65, q] PSUM tile).
  * Softmax normalization: the four denominator rows are broadcast by C=1
    matmuls packed two-per-PSUM-tile, inverted by two wide DVE reciprocals,
    then scaled; the whole chain is deferred into the next group's chunk
    loop (fires after its first scores/exps) so it overlaps useful work.
  * Projection (q/k/v) and output-projection pieces are queued as deferred
    closures and paced into the ACT-bound chunk loops where PE/DVE have
    slack; groups run largest-first so big loops absorb their neighbours'
    units and the tail group is the smallest.
  * bf16 matmul pipeline with fp32 PSUM; partial y outputs in bf16.
  * Inputs are host-packed [512,n] -> [128,4n] so every DMA is a plain 2D
    transfer with 2-14KB lines (3D-AP DMAs shred into sub-1KB packets at
    ~40% bandwidth); the first group's supply rides the two fast HWDGE
    rings (sync+scalar); y stores and o_tmp hops ride sync, not the slow
    gpsimd SWDGE ring.
"""

import ml_dtypes
import numpy as np

import concourse.bacc as bacc
import concourse.bass as bass
import concourse.mybir as mybir
from concourse import tile
from concourse.bass_utils import run_bass_kernel_spmd

F32 = mybir.dt.float32
F32R = mybir.dt.float32r
BF16 = mybir.dt.bfloat16

B, Q, KSEQ, D, H = 4, 2048, 2048, 512, 8
DH = D // H          # 64   head dim
QL = Q // 2          # 1024 queries per core
DL = 2 * DH          # 128  features per head-pair
NEG = -1.0e6
N_CORES = 8


def _pack4(a):
    """Host-side: [512, cols] -> [128, 4*cols] so device DMAs are plain 2D
    transfers with fat (>=2KB) per-partition lines (block ic at col ic*cols,
    matching the kernel's [:, ic*cols + j] slicing)."""
    return np.ascontiguousarray(
        a.reshape(4, 128, a.shape[1]).transpose(1, 0, 2).reshape(128, -1))


def build_nc(ktcs):
    """Single-core SPMD program; ktcs = per-batch key chunk counts (x128)."""
    assert len(ktcs) == B and all(1 <= c <= KSEQ // 128 for c in ktcs)
    EXP = mybir.ActivationFunctionType.Exp

    nc = bacc.Bacc("TRN2", target_bir_lowering=False, debug=False,
                   num_devices=N_CORES)

    def din(name, shape, dt=BF16):
        return nc.dram_tensor(name, shape, dt, kind="ExternalInput").ap()

    xq_d, xk_d, xv_d = [], [], []
    wq_d, wo_d, mask_d, y_d = [], [], [], []
    for g in range(B):
        kt = ktcs[g] * 128
        xq_d.append(din(f"xq{g}", [128, 4 * QL]))
        xk_d.append(din(f"xk{g}", [128, 4 * kt]))
        xv_d.append(din(f"xv{g}", [128, 4 * kt]))
        wq_d.append(din(f"w{g}", [128, 12 * DL]))  # wq|wk|wv packed
        wo_d.append(din(f"wo{g}", [DL, D]))
        mask_d.append(din(f"mask{g}", [128, ktcs[g]], F32))
        y_d.append(nc.dram_tensor(f"y{g}", [D, QL], BF16,
                                  kind="ExternalOutput").ap())

    with tile.TileContext(nc) as tc:
        with (
            nc.allow_low_precision(reason="bf16 matmul operands"),
            tc.tile_pool(name="persist", bufs=1) as pp,
            tc.tile_pool(name="cbuf", bufs=1) as cb,
        ):
            onescr = pp.tile([128, DH], F32, tag="onescr", name="onescr")
            nc.vector.memset(onescr[:], 1.0)
            ones_sb = pp.tile([65, DH], F32R, tag="ones", name="ones_sb")
            nc.vector.tensor_copy(ones_sb[64:65, :], onescr[64:65, :])
            _build_body(nc, tc, pp, cb, ones_sb, onescr, ktcs,
                        xq_d, xk_d, xv_d, wq_d, wo_d, mask_d, y_d)

    nc.compile()
    return nc


def _build_body(nc, tc, pp, cb, ones_sb, onescr, ktcs,
                xq_d, xk_d, xv_d, wq_d, wo_d, mask_d, y_d):
        EXP = mybir.ActivationFunctionType.Exp
        with (
            # PSUM: "s" tag 2x[128,1024] f32 (4 banks) shared by scores and
            # all projection outputs; 4 accumulator banks [65,512].
            tc.tile_pool(name="psS", bufs=2, space=bass.MemorySpace.PSUM) as psS,
            tc.tile_pool(name="psO", bufs=1, space=bass.MemorySpace.PSUM) as psO,
        ):
            # ---- input staging ----
            wqkv = [pp.tile([128, 12 * DL], BF16, tag=f"w{g}", name=f"w{g}")
                    for g in range(B)]
            wq = [wqkv[g][:, 0:4 * DL] for g in range(B)]
            wk = [wqkv[g][:, 4 * DL:8 * DL] for g in range(B)]
            wv = [wqkv[g][:, 8 * DL:12 * DL] for g in range(B)]
            wo = [pp.tile([DL, D], BF16, tag=f"wo{g}", name=f"wo{g}")
                  for g in range(B)]
            xq = [pp.tile([128, 4 * QL], BF16, tag=f"xq{g}", name=f"xq{g}")
                  for g in range(B)]
            xk = [pp.tile([128, 4 * ktcs[g] * 128], BF16, tag=f"xk{g}",
                          name=f"xk{g}") for g in range(B)]
            xv = [pp.tile([128, 4 * ktcs[g] * 128], BF16, tag=f"xv{g}",
                          name=f"xv{g}") for g in range(B)]
            mask_sb = [pp.tile([128, ktcs[g]], F32, tag=f"mask{g}",
                               name=f"mask{g}") for g in range(B)]

            # stage inputs in processing order (largest group first);
            # the first group's q path rides the scalar queue, which is
            # idle until the first exp, so qproj/kproj inputs land in
            # parallel instead of serializing on the sync queue
            order = sorted(range(B), key=lambda g: -ktcs[g])
            for gi, g in enumerate(order):
                if gi == 0:
                    # first group rides the two fast HWDGE rings so its
                    # chunk loop starts as early as possible; xv goes on
                    # gpsimd so the scalar (ACT) queue frees up for the
                    # exp table-load + first exps instead of streaming
                    # 1.75MB of V first (vproj only needs it ~1 chunk in)
                    nc.scalar.dma_start(wqkv[g][:], wq_d[g][:])
                    nc.scalar.dma_start(xq[g][:], xq_d[g][:])
                    nc.sync.dma_start(mask_sb[g][:], mask_d[g][:])
                    nc.sync.dma_start(xk[g][:], xk_d[g][:])
                    nc.gpsimd.dma_start(xv[g][:], xv_d[g][:])
                    nc.gpsimd.dma_start(wo[g][:], wo_d[g][:])
                else:
                    nc.sync.dma_start(wqkv[g][:], wq_d[g][:])
                    nc.sync.dma_start(xq[g][:], xq_d[g][:])
                    nc.sync.dma_start(mask_sb[g][:], mask_d[g][:])
                    nc.sync.dma_start(xk[g][:], xk_d[g][:])
                    _ = nc.gpsimd.dma_start(xv[g][:], xv_d[g][:])
                    nc.gpsimd.dma_start(wo[g][:], wo_d[g][:])

            q_t = [None] * B
            k_t = [None] * B
            v_sb = [None] * B

            # Deferred work units: projection pieces for the next group and
            # output-projection pieces for the previous group are drained
            # into the ACT-bound attention chunk loop, where the PE and DVE
            # have slack.  Each unit is a zero-arg closure.
            deferred = []  # entries: (kind, fn); kind 'qk' must finish
                           # before the next loop, 'v'/'wo' may spill

            def drain(n):
                for _ in range(min(n, len(deferred))):
                    deferred.pop(0)[1]()

            def drain_qk():
                while any(k == 'qk' for k, _ in deferred):
                    deferred.pop(0)[1]()

            def push_proj(g, eager_qk=False):
                KTC = ktcs[g]
                KT = KTC * 128

                def qalloc(g=g):
                    q_t[g] = pp.tile([128, QL], BF16, tag=f"q_t{g}",
                                     name=f"q_t{g}", uniquify=False)

                def qproj(qs, g=g):
                    ps = psS.tile([128, 512], F32, tag="s", name="ps")
                    for ic in range(4):
                        nc.tensor.matmul(
                            ps[:],
                            wq[g][:, ic * DL:(ic + 1) * DL],
                            xq[g][:, ic * QL + qs * 512:
                                  ic * QL + (qs + 1) * 512],
                            start=(ic == 0), stop=(ic == 3))
                    nc.vector.tensor_copy(
                        q_t[g][:, qs * 512:(qs + 1) * 512], ps[:])

                def kalloc(g=g, KT=KT):
                    k_t[g] = pp.tile([128, KT], BF16, tag=f"k_t{g}",
                                     name=f"k_t{g}", uniquify=False)

                def kproj(s, g=g, KT=KT):
                    w = min(1024, KT - s)
                    ps = psS.tile([128, 1024], F32, tag="s", name="ps")
                    for s2 in range(s, s + w, 512):
                        w2 = min(512, KT - s2)
                        for ic in range(4):
                            nc.tensor.matmul(
                                ps[:, s2 - s:s2 - s + w2],
                                wk[g][:, ic * DL:(ic + 1) * DL],
                                xk[g][:, ic * KT + s2:ic * KT + s2 + w2],
                                start=(ic == 0), stop=(ic == 3))
                    nc.vector.tensor_copy(k_t[g][:, s:s + w], ps[:, :w])

                def valloc(g=g, KTC=KTC):
                    v_sb[g] = [pp.tile([128, 2 * 65], BF16,
                                       tag=f"v{g}_{kt}", name=f"v{g}_{kt}",
                                       uniquify=False)
                               for kt in range(KTC)]

                def vproj(kt, g=g, KT=KT):
                    ps = psS.tile([128, DL], F32, tag="s", name="ps")
                    for ic in range(4):
                        nc.tensor.matmul(
                            ps[:],
                            xv[g][:, ic * KT + kt * 128:
                                  ic * KT + (kt + 1) * 128],
                            wv[g][:, ic * DL:(ic + 1) * DL],
                            start=(ic == 0), stop=(ic == 3))
                    nc.vector.memset(v_sb[g][kt][:], 1.0)
                    nc.vector.tensor_copy(
                        v_sb[g][kt].rearrange("p (h c) -> p h c", h=2)
                        [:, :, 0:64],
                        ps.rearrange("p (h c) -> p h c", h=2))

                qalloc()
                kalloc()
                valloc()
                if eager_qk:
                    for qs in range(2):
                        qproj(qs)
                    for s in range(0, KT, 1024):
                        kproj(s)
                else:
                    for qs in range(2):
                        deferred.append(('qk', lambda qs=qs: qproj(qs)))
                    for s in range(0, KT, 1024):
                        deferred.append(('qk', lambda s=s: kproj(s)))
                for kt in range(KTC):
                    deferred.append(('v', lambda kt=kt: vproj(kt)))

            def push_wo(g, o_pair):
                def wo_unit(ot, g=g, o_pair=o_pair):
                    y_ps = psS.tile([128, 1024], F32, tag="s", name="y_ps")
                    for qs in range(2):
                        nc.tensor.matmul(
                            y_ps[:, qs * 512:(qs + 1) * 512],
                            wo[g][:, ot * 128:(ot + 1) * 128],
                            o_pair[:, qs * 512:(qs + 1) * 512],
                            start=True, stop=True)
                    y_sb = cb.tile([128, 1024], BF16, tag="y", bufs=2,
                                   name="y_sb")
                    nc.vector.tensor_copy(y_sb[:], y_ps[:])
                    nc.sync.dma_start(
                        y_d[g][ot * 128:(ot + 1) * 128, :], y_sb[:])
                for ot in range(4):
                    deferred.append(('wo', lambda ot=ot: wo_unit(ot)))

            # process groups largest-first: big loops absorb the deferred
            # work of their neighbours and the tail group is the smallest
            push_proj(order[0], eager_qk=True)

            pending_norm = [None]

            for gi in range(B):
                g = order[gi]
                KTC = ktcs[g]
                if gi + 1 < B:
                    push_proj(order[gi + 1])

                o_ps = [[psO.tile([65, 512], F32, tag=f"o{h}{hf}",
                                  name=f"o{h}{hf}") for hf in range(2)]
                        for h in range(2)]

                def attnv(p01, kt, KTC=KTC, o_ps=o_ps, g=g):
                    for h in range(2):
                        for hf in range(2):
                            nc.tensor.matmul(
                                o_ps[h][hf][:],
                                v_sb[g][kt][:, h * 65:h * 65 + 65],
                                p01[hf][:, h * 512:(h + 1) * 512],
                                start=(kt == 0), stop=(kt == KTC - 1))

                prev = None
                for kt in range(KTC):
                    p01 = []
                    for hf in range(2):
                        s_ps = psS.tile([128, 1024], F32, tag="s",
                                        name="s_ps")
                        nc.tensor.matmul(
                            s_ps[:, 0:512],
                            k_t[g][0:64, kt * 128:(kt + 1) * 128],
                            q_t[g][0:64, hf * 512:(hf + 1) * 512],
                            start=True, stop=True)
                        nc.tensor.matmul(
                            s_ps[:, 512:1024],
                            k_t[g][64:128, kt * 128:(kt + 1) * 128],
                            q_t[g][64:128, hf * 512:(hf + 1) * 512],
                            start=True, stop=True)
                        p = cb.tile([128, 1024], BF16, tag="p", bufs=4,
                                    name="p")
                        nc.scalar.activation(p[:], s_ps[:], EXP,
                                             bias=mask_sb[g][:, kt:kt + 1],
                                             scale=1.0)
                        p01.append(p)
                    if kt == 0 and pending_norm[0] is not None:
                        pending_norm[0]()
                        pending_norm[0] = None
                    if prev is not None:
                        attnv(*prev)
                    # pace the deferred units evenly through the loop
                    if deferred:
                        drain(min(max(1, -(-len(deferred) // (KTC - kt))), 3))
                    prev = (p01, kt)
                attnv(*prev)
                drain_qk()

                # ---- normalize: o[dh, q] /= denom[q] (denom = row 64),
                # stage-batched, deferred into the next group's loop so the
                # chain overlaps its first scores/exps ----
                def norm(g=g, o_ps=o_ps):
                  o_pair = pp.tile([128, QL], BF16, tag=f"op{g}",
                                   name=f"o_pair{g}")
                  quads = [(h, hf) for h in range(2) for hf in range(2)]
                  dns = []
                  for h, hf in quads:
                      dn = cb.tile([65, 512], F32R, tag="dn", bufs=4,
                                   name="dn")
                      nc.vector.tensor_copy(dn[64:65, :],
                                            o_ps[h][hf][64:65, :])
                      dns.append(dn)
                  # denominator broadcasts pack 2-per-PSUM-tile (half the
                  # ring rotations, two wide reciprocals instead of four)
                  inv_sb = cb.tile([64, 2048], F32, tag="invb", bufs=2,
                                   name="inv_sb")
                  for j in range(2):
                      bc_ps = psS.tile([64, 1024], F32, tag="s",
                                       name="bc_ps")
                      for i in range(2):
                          nc.tensor.matmul(bc_ps[:, i * 512:(i + 1) * 512],
                                           ones_sb[64:65, :],
                                           dns[2 * j + i][64:65, :],
                                           start=True, stop=True)
                      nc.vector.reciprocal_approx_fast(
                          inv_sb[:, j * 1024:(j + 1) * 1024], bc_ps[:])
                  for i, (h, hf) in enumerate(quads):
                      ps = o_ps[h][hf]
                      inv = inv_sb[:, i * 512:(i + 1) * 512]
                      cols = slice(hf * 512, (hf + 1) * 512)
                      if h == 0:
                          nc.vector.tensor_mul(o_pair[0:64, cols],
                                               ps[0:64, :], inv)
                      else:
                          o_tmp = cb.tile([64, 512], BF16, tag="otmp",
                                          bufs=2, name="o_tmp")
                          nc.vector.tensor_mul(o_tmp[:], ps[0:64, :], inv)
                          nc.sync.dma_start(o_pair[64:128, cols],
                                            o_tmp[:])
                  # output projection drains into the following loop
                  push_wo(g, o_pair)

                pending_norm[0] = norm
            pending_norm[0]()
            drain(len(deferred))


def pick_ktcs(valid_lens):
    vl = np.asarray(valid_lens).astype(np.int64)
    return tuple(int(min(KSEQ // 128, max(1, (v + 127) // 128))) for v in vl)


def make_in_maps(queries, keys, values, valid_lens, W_q, W_k, W_v, W_o, ktcs):
    queries = np.asarray(queries, np.float32)
    keys = np.asarray(keys, np.float32)
    values = np.asarray(values, np.float32)
    W_q = np.asarray(W_q, np.float32)
    W_k = np.asarray(W_k, np.float32)
    W_v = np.asarray(W_v, np.float32)
    W_o = np.asarray(W_o, np.float32)
    vl = np.asarray(valid_lens).astype(np.int64)
    bf = ml_dtypes.bfloat16
    in_maps = []
    for c in range(N_CORES):
        hp, qh = c % 4, c // 4
        sl = slice(hp * DL, (hp + 1) * DL)
        m = {}
        for g in range(B):
            kt = ktcs[g] * 128
            mk = np.where(np.arange(kt) < vl[g], 0.0, NEG).astype(np.float32)
            m[f"xq{g}"] = _pack4(
                queries[g, qh * QL:(qh + 1) * QL].T.copy()).astype(bf)
            m[f"xk{g}"] = _pack4(keys[g, :kt].T.copy()).astype(bf)
            m[f"xv{g}"] = _pack4(values[g, :kt].T.copy()).astype(bf)
            m[f"w{g}"] = np.concatenate(
                [_pack4((W_q[sl, :] / 8.0).T.copy()),
                 _pack4(W_k[sl, :].T.copy()),
                 _pack4(W_v[sl, :].T.copy())], axis=1).astype(bf)
            m[f"wo{g}"] = np.ascontiguousarray(W_o[:, sl].T).astype(bf)
            m[f"mask{g}"] = np.ascontiguousarray(
                mk.reshape(ktcs[g], 128).T)
        in_maps.append(m)
    return in_maps


def gather_out(results):
    out = np.empty((B, Q, D), np.float32)
    for b in range(B):
        for qh in range(2):
            acc = np.zeros((D, QL), np.float32)
            for hp in range(4):
                acc += np.asarray(results[qh * 4 + hp][f"y{b}"],
                                  dtype=np.float32)
            out[b, qh * QL:(qh + 1) * QL] = acc.T
    return out


def kernel(queries, keys, values, valid_lens, W_q, W_k, W_v, W_o):
    ktcs = pick_ktcs(valid_lens)
    nc = build_nc(ktcs)
    in_maps = make_in_maps(queries, keys, values, valid_lens,
                           W_q, W_k, W_v, W_o, ktcs)
    res = run_bass_kernel_spmd(nc, in_maps, list(range(N_CORES))).results
    return gather_out(res)


# revision 37
# speedup vs baseline: 1.0429x; 1.0429x over previous
"""Multi-head attention (B=4, Q=K=2048, D=512, H=8) on 8 TRN2 NeuronCores.

Sharding: every core touches EVERY batch.  Core c owns head-pair hp = c%4
and query-half qh = c//4; its program runs 4 "groups", one per batch, and
group g's key extent is that batch's own ceil(valid_len/128)*128.  Per-core
work is therefore sum_b KT_b instead of 4*max_b KT_b -- perfectly balanced
across cores for any valid_lens.  Each group emits a partial output
projection (row-sharded W_o over its head pair); the host sums the 4
head-pair partials per (batch, query-half).

Device-side layout:
  * Activations live transposed ([feature, seq]); scores are computed
    transposed (S_T[k, q] = K_h @ Q_h^T) so the key-padding mask is a
    per-partition bias on the ACT exp and softmax needs no max pass.
  * The two heads of a pair sit at partitions 0-63 / 64-127 of shared
    q_t/k_t tiles, so their C=64 score matmuls are issued back-to-back and
    run CONCURRENTLY in disjoint PE row-groups (tile_position (0,0)/(64,0)
    auto-derived from base partitions).  Per chunk the scores land in two
    [128, 1024] PSUM tiles laid out [headA | headB] so each gets a single
    wide exp; the two tiles double-buffer in 4 PSUM banks (the other 4
    hold the attnV accumulators).
  * A ones-column interleaved into V makes the attnV matmul emit the
    softmax denominator for free (row 64 of each head's [65, q] PSUM tile).
  * Softmax normalization: the four denominator rows are broadcast by C=1
    matmuls packed two-per-PSUM-tile, inverted by two wide DVE reciprocals,
    then scaled; the whole chain is deferred into the next group's chunk
    loop (fires after its first scores/exps) so it overlaps useful work.
  * Projection (q/k/v) and output-projection pieces are queued as deferred
    closures and paced into the ACT-bound chunk loops where PE/DVE have
    slack; groups run largest-first so big loops absorb their neighbours'
    units and the tail group is the smallest.
  * bf16 matmul pipeline with fp32 PSUM; partial y outputs in bf16.
  * Inputs are host-packed [512,n] -> [128,4n] so every DMA is a plain 2D
    transfer with 2-14KB lines (3D-AP DMAs shred into sub-1KB packets at
    ~40% bandwidth); the first group's supply rides the two fast HWDGE
    rings (sync+scalar); y stores and o_tmp hops ride sync, not the slow
    gpsimd SWDGE ring.
"""

import ml_dtypes
import numpy as np

import concourse.bacc as bacc
import concourse.bass as bass
import concourse.mybir as mybir
from concourse import tile
from concourse.bass_utils import run_bass_kernel_spmd

F32 = mybir.dt.float32
F32R = mybir.dt.float32r
BF16 = mybir.dt.bfloat16

B, Q, KSEQ, D, H = 4, 2048, 2048, 512, 8
DH = D // H          # 64   head dim
QL = Q // 2          # 1024 queries per core
DL = 2 * DH          # 128  features per head-pair
NEG = -1.0e6
N_CORES = 8


def _pack4(a):
    """Host-side: [512, cols] -> [128, 4*cols] so device DMAs are plain 2D
    transfers with fat (>=2KB) per-partition lines (block ic at col ic*cols,
    matching the kernel's [:, ic*cols + j] slicing)."""
    return np.ascontiguousarray(
        a.reshape(4, 128, a.shape[1]).transpose(1, 0, 2).reshape(128, -1))


def build_nc(ktcs):
    """Single-core SPMD program; ktcs = per-batch key chunk counts (x128)."""
    assert len(ktcs) == B and all(1 <= c <= KSEQ // 128 for c in ktcs)
    EXP = mybir.ActivationFunctionType.Exp

    nc = bacc.Bacc("TRN2", target_bir_lowering=False, debug=False,
                   num_devices=N_CORES)

    def din(name, shape, dt=BF16):
        return nc.dram_tensor(name, shape, dt, kind="ExternalInput").ap()

    xq_d, xk_d, xv_d = [], [], []
    wq_d, wo_d, mask_d, y_d = [], [], [], []
    for g in range(B):
        kt = ktcs[g] * 128
        xq_d.append(din(f"xq{g}", [128, 4 * QL]))
        xk_d.append(din(f"xk{g}", [128, 4 * kt]))
        xv_d.append(din(f"xv{g}", [128, 4 * kt]))
        wq_d.append(din(f"w{g}", [128, 12 * DL]))  # wq|wk|wv packed
        wo_d.append(din(f"wo{g}", [DL, D]))
        mask_d.append(din(f"mask{g}", [128, ktcs[g]], F32))
        y_d.append(nc.dram_tensor(f"y{g}", [D, QL], BF16,
                                  kind="ExternalOutput").ap())

    with tile.TileContext(nc) as tc:
        with (
            nc.allow_low_precision(reason="bf16 matmul operands"),
            tc.tile_pool(name="persist", bufs=1) as pp,
            tc.tile_pool(name="cbuf", bufs=1) as cb,
        ):
            onescr = pp.tile([128, DH], F32, tag="onescr", name="onescr")
            nc.vector.memset(onescr[:], 1.0)
            ones_sb = pp.tile([65, DH], F32R, tag="ones", name="ones_sb")
            nc.vector.tensor_copy(ones_sb[64:65, :], onescr[64:65, :])
            _build_body(nc, tc, pp, cb, ones_sb, onescr, ktcs,
                        xq_d, xk_d, xv_d, wq_d, wo_d, mask_d, y_d)

    nc.compile()
    return nc


def _build_body(nc, tc, pp, cb, ones_sb, onescr, ktcs,
                xq_d, xk_d, xv_d, wq_d, wo_d, mask_d, y_d):
        EXP = mybir.ActivationFunctionType.Exp
        with (
            # PSUM: "s" tag 2x[128,1024] f32 (4 banks) shared by scores and
            # all projection outputs; 4 accumulator banks [65,512].
            tc.tile_pool(name="psS", bufs=2, space=bass.MemorySpace.PSUM) as psS,
            tc.tile_pool(name="psO", bufs=1, space=bass.MemorySpace.PSUM) as psO,
        ):
            # ---- input staging ----
            wqkv = [pp.tile([128, 12 * DL], BF16, tag=f"w{g}", name=f"w{g}")
                    for g in range(B)]
            wq = [wqkv[g][:, 0:4 * DL] for g in range(B)]
            wk = [wqkv[g][:, 4 * DL:8 * DL] for g in range(B)]
            wv = [wqkv[g][:, 8 * DL:12 * DL] for g in range(B)]
            wo = [pp.tile([DL, D], BF16, tag=f"wo{g}", name=f"wo{g}")
                  for g in range(B)]
            xq = [pp.tile([128, 4 * QL], BF16, tag=f"xq{g}", name=f"xq{g}")
                  for g in range(B)]
            xk = [pp.tile([128, 4 * ktcs[g] * 128], BF16, tag=f"xk{g}",
                          name=f"xk{g}") for g in range(B)]
            xv = [pp.tile([128, 4 * ktcs[g] * 128], BF16, tag=f"xv{g}",
                          name=f"xv{g}") for g in range(B)]
            mask_sb = [pp.tile([128, ktcs[g]], F32, tag=f"mask{g}",
                               name=f"mask{g}") for g in range(B)]

            # stage inputs in processing order (largest group first);
            # the first group's q path rides the scalar queue, which is
            # idle until the first exp, so qproj/kproj inputs land in
            # parallel instead of serializing on the sync queue
            order = sorted(range(B), key=lambda g: -ktcs[g])
            for gi, g in enumerate(order):
                if gi == 0:
                    # first group rides the two fast HWDGE rings so its
                    # chunk loop starts as early as possible
                    nc.scalar.dma_start(wqkv[g][:], wq_d[g][:])
                    nc.scalar.dma_start(xq[g][:], xq_d[g][:])
                    nc.sync.dma_start(mask_sb[g][:], mask_d[g][:])
                    nc.sync.dma_start(xk[g][:], xk_d[g][:])
                    nc.scalar.dma_start(xv[g][:], xv_d[g][:])
                    nc.gpsimd.dma_start(wo[g][:], wo_d[g][:])
                else:
                    nc.sync.dma_start(wqkv[g][:], wq_d[g][:])
                    nc.sync.dma_start(xq[g][:], xq_d[g][:])
                    nc.sync.dma_start(mask_sb[g][:], mask_d[g][:])
                    nc.sync.dma_start(xk[g][:], xk_d[g][:])
                    _ = nc.gpsimd.dma_start(xv[g][:], xv_d[g][:])
                    nc.gpsimd.dma_start(wo[g][:], wo_d[g][:])

            q_t = [None] * B
            k_t = [None] * B
            v_sb = [None] * B

            # Deferred work units: projection pieces for the next group and
            # output-projection pieces for the previous group are drained
            # into the ACT-bound attention chunk loop, where the PE and DVE
            # have slack.  Each unit is a zero-arg closure.
            deferred = []  # entries: (kind, fn); kind 'qk' must finish
                           # before the next loop, 'v'/'wo' may spill

            def drain(n):
                for _ in range(min(n, len(deferred))):
                    deferred.pop(0)[1]()

            def drain_qk():
                while any(k == 'qk' for k, _ in deferred):
                    deferred.pop(0)[1]()

            def push_proj(g, eager_qk=False):
                KTC = ktcs[g]
                KT = KTC * 128

                def qalloc(g=g):
                    q_t[g] = pp.tile([128, QL], BF16, tag=f"q_t{g}",
                                     name=f"q_t{g}", uniquify=False)

                def qproj(qs, g=g):
                    ps = psS.tile([128, 512], F32, tag="s", name="ps")
                    for ic in range(4):
                        nc.tensor.matmul(
                            ps[:],
                            wq[g][:, ic * DL:(ic + 1) * DL],
                            xq[g][:, ic * QL + qs * 512:
                                  ic * QL + (qs + 1) * 512],
                            start=(ic == 0), stop=(ic == 3))
                    nc.vector.tensor_copy(
                        q_t[g][:, qs * 512:(qs + 1) * 512], ps[:])

                def kalloc(g=g, KT=KT):
                    k_t[g] = pp.tile([128, KT], BF16, tag=f"k_t{g}",
                                     name=f"k_t{g}", uniquify=False)

                def kproj(s, g=g, KT=KT):
                    w = min(1024, KT - s)
                    ps = psS.tile([128, 1024], F32, tag="s", name="ps")
                    for s2 in range(s, s + w, 512):
                        w2 = min(512, KT - s2)
                        for ic in range(4):
                            nc.tensor.matmul(
                                ps[:, s2 - s:s2 - s + w2],
                                wk[g][:, ic * DL:(ic + 1) * DL],
                                xk[g][:, ic * KT + s2:ic * KT + s2 + w2],
                                start=(ic == 0), stop=(ic == 3))
                    nc.vector.tensor_copy(k_t[g][:, s:s + w], ps[:, :w])

                def valloc(g=g, KTC=KTC):
                    v_sb[g] = [pp.tile([128, 2 * 65], BF16,
                                       tag=f"v{g}_{kt}", name=f"v{g}_{kt}",
                                       uniquify=False)
                               for kt in range(KTC)]

                def vproj(kt, g=g, KT=KT):
                    ps = psS.tile([128, DL], F32, tag="s", name="ps")
                    for ic in range(4):
                        nc.tensor.matmul(
                            ps[:],
                            xv[g][:, ic * KT + kt * 128:
                                  ic * KT + (kt + 1) * 128],
                            wv[g][:, ic * DL:(ic + 1) * DL],
                            start=(ic == 0), stop=(ic == 3))
                    nc.vector.memset(v_sb[g][kt][:], 1.0)
                    nc.vector.tensor_copy(
                        v_sb[g][kt].rearrange("p (h c) -> p h c", h=2)
                        [:, :, 0:64],
                        ps.rearrange("p (h c) -> p h c", h=2))

                qalloc()
                kalloc()
                valloc()
                if eager_qk:
                    for qs in range(2):
                        qproj(qs)
                    for s in range(0, KT, 1024):
                        kproj(s)
                else:
                    for qs in range(2):
                        deferred.append(('qk', lambda qs=qs: qproj(qs)))
                    for s in range(0, KT, 1024):
                        deferred.append(('qk', lambda s=s: kproj(s)))
                for kt in range(KTC):
                    deferred.append(('v', lambda kt=kt: vproj(kt)))

            def push_wo(g, o_pair):
                def wo_unit(ot, g=g, o_pair=o_pair):
                    y_ps = psS.tile([128, 1024], F32, tag="s", name="y_ps")
                    for qs in range(2):
                        nc.tensor.matmul(
                            y_ps[:, qs * 512:(qs + 1) * 512],
                            wo[g][:, ot * 128:(ot + 1) * 128],
                            o_pair[:, qs * 512:(qs + 1) * 512],
                            start=True, stop=True)
                    y_sb = cb.tile([128, 1024], BF16, tag="y", bufs=2,
                                   name="y_sb")
                    nc.vector.tensor_copy(y_sb[:], y_ps[:])
                    nc.sync.dma_start(
                        y_d[g][ot * 128:(ot + 1) * 128, :], y_sb[:])
                for ot in range(4):
                    deferred.append(('wo', lambda ot=ot: wo_unit(ot)))

            # process groups largest-first: big loops absorb the deferred
            # work of their neighbours and the tail group is the smallest
            push_proj(order[0], eager_qk=True)

            pending_norm = [None]

            for gi in range(B):
                g = order[gi]
                KTC = ktcs[g]
                if gi + 1 < B:
                    push_proj(order[gi + 1])

                o_ps = [[psO.tile([65, 512], F32, tag=f"o{h}{hf}",
                                  name=f"o{h}{hf}") for hf in range(2)]
                        for h in range(2)]

                def attnv(p01, kt, KTC=KTC, o_ps=o_ps, g=g):
                    for h in range(2):
                        for hf in range(2):
                            nc.tensor.matmul(
                                o_ps[h][hf][:],
                                v_sb[g][kt][:, h * 65:h * 65 + 65],
                                p01[hf][:, h * 512:(h + 1) * 512],
                                start=(kt == 0), stop=(kt == KTC - 1))

                prev = None
                for kt in range(KTC):
                    p01 = []
                    for hf in range(2):
                        s_ps = psS.tile([128, 1024], F32, tag="s",
                                        name="s_ps")
                        nc.tensor.matmul(
                            s_ps[:, 0:512],
                            k_t[g][0:64, kt * 128:(kt + 1) * 128],
                            q_t[g][0:64, hf * 512:(hf + 1) * 512],
                            start=True, stop=True)
                        nc.tensor.matmul(
                            s_ps[:, 512:1024],
                            k_t[g][64:128, kt * 128:(kt + 1) * 128],
                            q_t[g][64:128, hf * 512:(hf + 1) * 512],
                            start=True, stop=True)
                        p = cb.tile([128, 1024], BF16, tag="p", bufs=4,
                                    name="p")
                        nc.scalar.activation(p[:], s_ps[:], EXP,
                                             bias=mask_sb[g][:, kt:kt + 1],
                                             scale=1.0)
                        p01.append(p)
                    if kt == 0 and pending_norm[0] is not None:
                        pending_norm[0]()
                        pending_norm[0] = None
                    if prev is not None:
                        attnv(*prev)
                    # pace the deferred units evenly through the loop
                    if deferred:
                        drain(min(max(1, -(-len(deferred) // (KTC - kt))), 3))
                    prev = (p01, kt)
                attnv(*prev)
                drain_qk()

                # ---- normalize: o[dh, q] /= denom[q] (denom = row 64),
                # stage-batched, deferred into the next group's loop so the
                # chain overlaps its first scores/exps ----
                def norm(g=g, o_ps=o_ps):
                  o_pair = pp.tile([128, QL], BF16, tag=f"op{g}",
                                   name=f"o_pair{g}")
                  quads = [(h, hf) for h in range(2) for hf in range(2)]
                  dns = []
                  for h, hf in quads:
                      dn = cb.tile([65, 512], F32R, tag="dn", bufs=4,
                                   name="dn")
                      nc.vector.tensor_copy(dn[64:65, :],
                                            o_ps[h][hf][64:65, :])
                      dns.append(dn)
                  # denominator broadcasts pack 2-per-PSUM-tile (half the
                  # ring rotations, two wide reciprocals instead of four)
                  inv_sb = cb.tile([64, 2048], F32, tag="invb", bufs=2,
                                   name="inv_sb")
                  for j in range(2):
                      bc_ps = psS.tile([64, 1024], F32, tag="s",
                                       name="bc_ps")
                      for i in range(2):
                          nc.tensor.matmul(bc_ps[:, i * 512:(i + 1) * 512],
                                           ones_sb[64:65, :],
                                           dns[2 * j + i][64:65, :],
                                           start=True, stop=True)
                      nc.vector.reciprocal_approx_fast(
                          inv_sb[:, j * 1024:(j + 1) * 1024], bc_ps[:])
                  for i, (h, hf) in enumerate(quads):
                      ps = o_ps[h][hf]
                      inv = inv_sb[:, i * 512:(i + 1) * 512]
                      cols = slice(hf * 512, (hf + 1) * 512)
                      if h == 0:
                          nc.vector.tensor_mul(o_pair[0:64, cols],
                                               ps[0:64, :], inv)
                      else:
                          o_tmp = cb.tile([64, 512], BF16, tag="otmp",
                                          bufs=2, name="o_tmp")
                          nc.vector.tensor_mul(o_tmp[:], ps[0:64, :], inv)
                          nc.sync.dma_start(o_pair[64:128, cols],
                                            o_tmp[:])
                  # output projection drains into the following loop
                  push_wo(g, o_pair)

                pending_norm[0] = norm
            pending_norm[0]()
            drain(len(deferred))


def pick_ktcs(valid_lens):
    vl = np.asarray(valid_lens).astype(np.int64)
    return tuple(int(min(KSEQ // 128, max(1, (v + 127) // 128))) for v in vl)


def make_in_maps(queries, keys, values, valid_lens, W_q, W_k, W_v, W_o, ktcs):
    queries = np.asarray(queries, np.float32)
    keys = np.asarray(keys, np.float32)
    values = np.asarray(values, np.float32)
    W_q = np.asarray(W_q, np.float32)
    W_k = np.asarray(W_k, np.float32)
    W_v = np.asarray(W_v, np.float32)
    W_o = np.asarray(W_o, np.float32)
    vl = np.asarray(valid_lens).astype(np.int64)
    bf = ml_dtypes.bfloat16
    in_maps = []
    for c in range(N_CORES):
        hp, qh = c % 4, c // 4
        sl = slice(hp * DL, (hp + 1) * DL)
        m = {}
        for g in range(B):
            kt = ktcs[g] * 128
            mk = np.where(np.arange(kt) < vl[g], 0.0, NEG).astype(np.float32)
            m[f"xq{g}"] = _pack4(
                queries[g, qh * QL:(qh + 1) * QL].T.copy()).astype(bf)
            m[f"xk{g}"] = _pack4(keys[g, :kt].T.copy()).astype(bf)
            m[f"xv{g}"] = _pack4(values[g, :kt].T.copy()).astype(bf)
            m[f"w{g}"] = np.concatenate(
                [_pack4((W_q[sl, :] / 8.0).T.copy()),
                 _pack4(W_k[sl, :].T.copy()),
                 _pack4(W_v[sl, :].T.copy())], axis=1).astype(bf)
            m[f"wo{g}"] = np.ascontiguousarray(W_o[:, sl].T).astype(bf)
            m[f"mask{g}"] = np.ascontiguousarray(
                mk.reshape(ktcs[g], 128).T)
        in_maps.append(m)
    return in_maps


def gather_out(results):
    out = np.empty((B, Q, D), np.float32)
    for b in range(B):
        for qh in range(2):
            acc = np.zeros((D, QL), np.float32)
            for hp in range(4):
                acc += np.asarray(results[qh * 4 + hp][f"y{b}"],
                                  dtype=np.float32)
            out[b, qh * QL:(qh + 1) * QL] = acc.T
    return out


def kernel(queries, keys, values, valid_lens, W_q, W_k, W_v, W_o):
    ktcs = pick_ktcs(valid_lens)
    nc = build_nc(ktcs)
    in_maps = make_in_maps(queries, keys, values, valid_lens,
                           W_q, W_k, W_v, W_o, ktcs)
    res = run_bass_kernel_spmd(nc, in_maps, list(range(N_CORES))).results
    return gather_out(res)
